# revision 6
# baseline (speedup 1.0000x reference)
"""Causal self-attention TRN2 kernel (8 NeuronCores, Megatron-style sharding).

Reference computation (fp32):
    qkv = x @ w_attn.T ; q,k,v split; per-head causal softmax(q k^T/sqrt(hs)) v
    out = y @ w_proj.T
Shapes: x [4, 2048, 1024], w_attn [3072, 1024], w_proj [1024, 1024], 16 heads.

Sharding: core = (b, g) with b = batch 0..3, g = head-group 0..1 (8 heads each).
Each core computes its batch's attention for its 8 heads plus the partial
output projection over its 512 local head-dims; host sums the two partials
per batch (Megatron row-parallel) and transposes back.

Device dataflow is fully transposed ([feature, token] layout) so the PE
contraction dim always sits on partitions with zero on-device transposes:
  qkT[d, t] = waT.T @ xT           (lhsT = waT block, rhs = xT)
  V[t, d]   = xT.T @ waT_v         (lhsT = xT block, rhs = wv)
  S.T[k, q] = KT.T @ QT            (lhsT = KT slice [hs=64, 128], 2 heads share
                                    the PE via row groups 0-1 / 2-3)
  P = exp(S/8) with causal 0/1 mask applied after exp (values are bounded, so
      no max-subtraction is needed); denominators come free from a ones column
      appended to V (y.T matmul has M=65, row 64 = sum_k P)
  y.T[d, q] = V_aug.T @ P          (accumulated over k-tiles in PSUM)
  outT[e, q] = wpT.T @ yT          (partial over local d)
All matmuls run in float32r (full PE rate at N=512, ~1e-4 relative error).
"""

import math

import numpy as np

import concourse.bass as bass
import concourse.tile as tile
from concourse import bacc, mybir
from concourse import bass_utils

F32R = mybir.dt.float32r
F32 = mybir.dt.float32

C = 1024          # embed dim
NH_LOCAL = 8      # heads per core
HS = 64           # head size
DL = NH_LOCAL * HS  # local head-dim total (512)
NCT = C // 128    # c-tiles (contraction tiles) = 8


def build(T: int = 2048):
    """Build + compile the per-core program for sequence length T."""
    NQC = T // 512    # q-chunks
    NKT = T // 128    # k-tiles / t-tiles

    nc = bacc.Bacc(
        "TRN2", target_bir_lowering=False, debug=False, enable_asserts=False
    )

    xT = nc.dram_tensor("xT", [C, T], F32R, kind="ExternalInput").ap()
    waT = nc.dram_tensor("waT", [C, 3 * DL], F32R, kind="ExternalInput").ap()
    wpT = nc.dram_tensor("wpT", [DL, C], F32R, kind="ExternalInput").ap()
    masks = nc.dram_tensor("masks", [4, 128, 512], F32R, kind="ExternalInput").ap()
    outT = nc.dram_tensor("outT", [C, T], F32, kind="ExternalOutput").ap()

    with tile.TileContext(nc) as tc:
        with (
            tc.tile_pool(name="const", bufs=1) as constp,
            tc.tile_pool(name="persist", bufs=1) as persist,
            tc.tile_pool(name="dram", bufs=1, space="DRAM") as dramp,
            # xT (phase A) and QT/KT streams (phase B) share these 8 slots
            tc.tile_pool(name="big", bufs=8) as bigp,
            tc.tile_pool(name="wblk", bufs=16) as wblkp,
            tc.tile_pool(name="wv", bufs=8) as wvp,
            tc.tile_pool(name="wpe", bufs=2) as wpep,
            tc.tile_pool(name="stage", bufs=4) as stagep,
            tc.tile_pool(name="epool", bufs=4) as epool,
            tc.tile_pool(name="misc", bufs=2) as miscp,
            tc.tile_pool(name="ps_small", bufs=2, space="PSUM") as ps_small,
            tc.tile_pool(name="ps_st", bufs=3, space="PSUM") as ps_st,
            tc.tile_pool(name="ps_yt", bufs=3, space="PSUM") as ps_yt,
        ):
            # ---- constants ----
            mask_t = []
            for o in range(4):
                mt = constp.tile([128, 512], F32R, tag=f"mask{o}", name=f"mask{o}")
                nc.sync.dma_start(mt[:], masks[o])
                mask_t.append(mt)

            # ---- persistent activations ----
            va_t = []  # V augmented with ones column: [128, 8*65]
            for tt in range(NKT):
                va = persist.tile(
                    [128, NH_LOCAL * (HS + 1)], F32R, tag=f"va{tt}", name=f"va{tt}"
                )
                va_t.append(va)
            yt_t = []  # y.T per head-pair: [128, T]
            for p in range(4):
                yt = persist.tile([128, T], F32R, tag=f"yt{p}", name=f"yt{p}")
                yt_t.append(yt)

            qk_dram = dramp.tile([2 * DL, T], F32R, tag="qk_dram", name="qk_dram")

            # ================= phase A: projections =================
            xt_t = []
            for ci in range(NCT):
                xt = bigp.tile([128, T], F32R, tag="big", name=f"xt{ci}")
                nc.sync.dma_start(xt[:], xT[128 * ci : 128 * (ci + 1), :])
                xt_t.append(xt)

            # qkT = waT.T @ xT, written to DRAM bounce buffer
            for dt in range(2 * DL // 128):  # 8 d-tiles: 4 q-pairs then 4 k-pairs
                wbs = []
                for ci in range(NCT):
                    wb = wblkp.tile([128, 128], F32R, tag="wblk", name=f"wb{dt}_{ci}")
                    nc.sync.dma_start(
                        wb[:],
                        waT[128 * ci : 128 * (ci + 1), 128 * dt : 128 * (dt + 1)],
                    )
                    wbs.append(wb)
                for jq in range(T // 512):
                    ps = ps_small.tile([128, 512], F32, tag="psA", name="psA")
                    for ci in range(NCT):
                        nc.tensor.matmul(
                            ps[:],
                            wbs[ci][:],
                            xt_t[ci][:, 512 * jq : 512 * (jq + 1)],
                            start=(ci == 0),
                            stop=(ci == NCT - 1),
                        )
                    st = stagep.tile([128, 512], F32R, tag="stage", name="stA")
                    nc.vector.tensor_copy(st[:], ps[:])
                    nc.sync.dma_start(
                        qk_dram[128 * dt : 128 * (dt + 1), 512 * jq : 512 * (jq + 1)],
                        st[:],
                    )

            # V = xT.T @ wv  (+ ones column per head)
            wv_t = []
            for ci in range(NCT):
                wv = wvp.tile([128, DL], F32R, tag="wv", name=f"wv{ci}")
                nc.sync.dma_start(wv[:], waT[128 * ci : 128 * (ci + 1), 2 * DL :])
                wv_t.append(wv)
            for tt in range(NKT):
                ps = ps_small.tile([128, 512], F32, tag="psA", name="psV")
                for ci in range(NCT):
                    nc.tensor.matmul(
                        ps[:],
                        xt_t[ci][:, 128 * tt : 128 * (tt + 1)],
                        wv_t[ci][:],
                        start=(ci == 0),
                        stop=(ci == NCT - 1),
                    )
                va = va_t[tt]
                va3 = va[:].rearrange("p (h d) -> p h d", d=HS + 1)
                ps3 = ps[:].rearrange("p (h d) -> p h d", d=HS)
                nc.vector.tensor_copy(va3[:, :, 0:HS], ps3[:])
                nc.vector.memset(va3[:, :, HS].bitcast(F32), 1.0)

            # ================= phase B: attention =================
            for p in range(4):  # head pairs
                qt = bigp.tile([128, T], F32R, tag="big", name=f"qt{p}")
                nc.sync.dma_start(qt[:], qk_dram[128 * p : 128 * (p + 1), :])
                kt = bigp.tile([128, T], F32R, tag="big", name=f"kt{p}")
                nc.sync.dma_start(kt[:], qk_dram[DL + 128 * p : DL + 128 * (p + 1), :])

                for j in range(NQC):
                    qs = slice(512 * j, 512 * (j + 1))
                    ytps = [
                        ps_yt.tile([HS + 1, 512], F32, tag="ytp", name="ytp0"),
                        ps_yt.tile([HS + 1, 512], F32, tag="ytp", name="ytp1"),
                    ]
                    n_kt = 4 * j + 4
                    for i in range(n_kt):
                        ks = slice(128 * i, 128 * (i + 1))
                        for h in range(2):  # head within pair, row-group packed
                            hp = slice(64 * h, 64 * (h + 1))
                            st = ps_st.tile([128, 512], F32, tag="stp", name="stp")
                            nc.tensor.matmul(
                                st[:], kt[hp, ks], qt[hp, qs], start=True, stop=True
                            )
                            et = epool.tile([128, 512], F32R, tag="et", name="et")
                            nc.scalar.activation(
                                et[:],
                                st[:],
                                mybir.ActivationFunctionType.Exp,
                                scale=1.0 / math.sqrt(HS),
                            )
                            if i >= 4 * j:  # diagonal-crossing tile: causal mask
                                nc.vector.tensor_mul(
                                    et[:], et[:], mask_t[i - 4 * j][:]
                                )
                            hh = 2 * p + h
                            nc.tensor.matmul(
                                ytps[h][:],
                                va_t[i][:, 65 * hh : 65 * hh + 65],
                                et[:],
                                start=(i == 0),
                                stop=(i == n_kt - 1),
                            )
                    for h in range(2):
                        rc = miscp.tile([1, 512], F32, tag="rc", name="rc")
                        nc.vector.reciprocal(rc[:], ytps[h][HS : HS + 1, :])
                        rb = miscp.tile([64, 512], F32, tag="rb", name="rb")
                        nc.gpsimd.partition_broadcast(rb[:], rc[:])
                        nc.vector.tensor_mul(
                            yt_t[p][64 * h : 64 * (h + 1), qs], ytps[h][0:HS, :], rb[:]
                        )

            # ================= phase C: output projection =================
            for e in range(C // 128):
                wps = []
                for p in range(4):
                    wp = wpep.tile([128, 128], F32R, tag=f"wpe{p}", name=f"wpe{p}")
                    nc.sync.dma_start(
                        wp[:],
                        wpT[128 * p : 128 * (p + 1), 128 * e : 128 * (e + 1)],
                    )
                    wps.append(wp)
                for jq in range(T // 512):
                    ps = ps_small.tile([128, 512], F32, tag="psA", name="psC")
                    for p in range(4):
                        nc.tensor.matmul(
                            ps[:],
                            wps[p][:],
                            yt_t[p][:, 512 * jq : 512 * (jq + 1)],
                            start=(p == 0),
                            stop=(p == 3),
                        )
                    ot = stagep.tile([128, 512], F32, tag="stage", name="stC")
                    nc.scalar.copy(ot[:], ps[:])
                    nc.sync.dma_start(
                        outT[128 * e : 128 * (e + 1), 512 * jq : 512 * (jq + 1)],
                        ot[:],
                    )

    nc.compile()
    return nc


_CACHE: dict = {}
_LAST_IN_MAPS = None


def _get_nc(T: int):
    if T not in _CACHE:
        _CACHE[T] = build(T)
    return _CACHE[T]


def _make_masks() -> np.ndarray:
    kk = np.arange(128)[:, None]
    qq = np.arange(512)[None, :]
    return np.stack(
        [(qq >= 128 * o + kk).astype(np.float32) for o in range(4)]
    )


def kernel(x: np.ndarray, w_attn: np.ndarray, w_proj: np.ndarray) -> np.ndarray:
    B, T, C_ = x.shape
    nc = _get_nc(T)
    masks = _make_masks()

    in_maps = []
    for core in range(8):
        b, g = core // 2, core % 2
        heads = range(8 * g, 8 * g + 8)
        rows = []
        for base in (0, C_, 2 * C_):  # q, k, v sections of w_attn
            for H in heads:
                rows.extend(range(base + 64 * H, base + 64 * H + 64))
        waT_l = np.ascontiguousarray(np.asarray(w_attn)[rows, :].T.astype(np.float32))
        dcols = [c for H in heads for c in range(64 * H, 64 * H + 64)]
        wpT_l = np.ascontiguousarray(np.asarray(w_proj)[:, dcols].T.astype(np.float32))
        xT_l = np.ascontiguousarray(np.asarray(x[b]).T.astype(np.float32))
        in_maps.append({"xT": xT_l, "waT": waT_l, "wpT": wpT_l, "masks": masks})

    global _LAST_IN_MAPS
    _LAST_IN_MAPS = in_maps
    res = bass_utils.run_bass_kernel_spmd(nc, in_maps, core_ids=list(range(8)))
    out = np.empty((B, T, C_), dtype=np.float32)
    for b in range(B):
        out[b] = (
            res.results[2 * b]["outT"].astype(np.float32)
            + res.results[2 * b + 1]["outT"].astype(np.float32)
        ).T
    return out


# revision 16
# speedup vs baseline: 1.0899x; 1.0899x over previous
"""Causal self-attention TRN2 kernel (8 NeuronCores, Megatron-style sharding).

Reference computation (fp32):
    qkv = x @ w_attn.T ; q,k,v split; per-head causal softmax(q k^T/sqrt(hs)) v
    out = y @ w_proj.T
Shapes: x [4, 2048, 1024], w_attn [3072, 1024], w_proj [1024, 1024], 16 heads.

Sharding: core = (b, g) with b = batch 0..3, g = head-group 0..1 (8 heads each).
Each core computes its batch's attention for its 8 heads plus the partial
output projection over its 512 local head-dims; host sums the two partials
per batch (Megatron row-parallel) and transposes back.

Device dataflow is fully transposed ([feature, token] layout) so the PE
contraction dim always sits on partitions with zero on-device transposes:
  qkT[d, t] = waT.T @ xT           (lhsT = waT block, rhs = xT)
  V[t, d]   = xT.T @ waT_v         (lhsT = xT block, rhs = wv)
  S.T[k, q] = KT.T @ QT            (lhsT = KT slice [hs=64, 128], 2 heads share
                                    the PE via row groups 0-1 / 2-3)
  P = exp(S/8) with causal 0/1 mask applied after exp (values are bounded, so
      no max-subtraction is needed); denominators come free from a ones column
      appended to V (y.T matmul has M=65, row 64 = sum_k P)
  y.T[d, q] = V_aug.T @ P          (accumulated over k-tiles in PSUM)
  outT[e, q] = wpT.T @ yT          (partial over local d)
All matmuls run in float32r (full PE rate at N=512, ~1e-4 relative error).
"""

import math

import numpy as np

import concourse.bass as bass
import concourse.tile as tile
from concourse import bacc, mybir
from concourse import bass_utils

F32R = mybir.dt.float32r
F32 = mybir.dt.float32

C = 1024          # embed dim
NH_LOCAL = 8      # heads per core
HS = 64           # head size
DL = NH_LOCAL * HS  # local head-dim total (512)
NCT = C // 128    # c-tiles (contraction tiles) = 8


def build(T: int = 2048):
    """Build + compile the per-core program for sequence length T."""
    NQC = T // 512    # q-chunks
    NKT = T // 128    # k-tiles / t-tiles

    nc = bacc.Bacc(
        "TRN2", target_bir_lowering=False, debug=False, enable_asserts=False
    )

    xT = nc.dram_tensor("xT", [C, T], F32R, kind="ExternalInput").ap()
    waT = nc.dram_tensor("waT", [C, 3 * DL], F32R, kind="ExternalInput").ap()
    wpT = nc.dram_tensor("wpT", [DL, C], F32R, kind="ExternalInput").ap()
    masks = nc.dram_tensor("masks", [4, 128, 512], F32R, kind="ExternalInput").ap()
    outT = nc.dram_tensor("outT", [C, T], F32, kind="ExternalOutput").ap()

    with tile.TileContext(nc) as tc:
        with (
            tc.tile_pool(name="const", bufs=1) as constp,
            tc.tile_pool(name="persist", bufs=1) as persist,
            tc.tile_pool(name="dram", bufs=1, space="DRAM") as dramp,
            # xT (phase A) and QT/KT streams (phase B) share these slots; the
            # 2 extra slots let pair-0's QT/KT prefetch while phase A runs
            tc.tile_pool(name="big", bufs=10) as bigp,
            tc.tile_pool(name="wblk", bufs=12) as wblkp,
            tc.tile_pool(name="wv", bufs=8) as wvp,
            tc.tile_pool(name="wpe", bufs=2) as wpep,
            tc.tile_pool(name="stage", bufs=4) as stagep,
            tc.tile_pool(name="epool", bufs=4) as epool,
            tc.tile_pool(name="misc", bufs=2) as miscp,
            tc.tile_pool(name="ps_small", bufs=2, space="PSUM") as ps_small,
            tc.tile_pool(name="ps_st", bufs=3, space="PSUM") as ps_st,
            tc.tile_pool(name="ps_yt", bufs=3, space="PSUM") as ps_yt,
        ):
            # ---- constants ----
            mask_t = []
            for o in range(4):
                mt = constp.tile([128, 512], F32R, tag=f"mask{o}", name=f"mask{o}")
                nc.sync.dma_start(mt[:], masks[o])
                mask_t.append(mt)

            # ---- persistent activations ----
            va_t = []  # V augmented with ones column: [128, 8*65]
            for tt in range(NKT):
                va = persist.tile(
                    [128, NH_LOCAL * (HS + 1)], F32R, tag=f"va{tt}", name=f"va{tt}"
                )
                va_t.append(va)
            yt_t = []  # y.T per head-pair: [128, T]
            for p in range(4):
                yt = persist.tile([128, T], F32R, tag=f"yt{p}", name=f"yt{p}")
                yt_t.append(yt)

            qk_dram = dramp.tile([2 * DL, T], F32R, tag="qk_dram", name="qk_dram")

            # ================= phase A: projections =================
            xt_t = []
            for ci in range(NCT):
                xt = bigp.tile([128, T], F32R, tag="big", name=f"xt{ci}")
                nc.sync.dma_start(xt[:], xT[128 * ci : 128 * (ci + 1), :])
                xt_t.append(xt)

            # qkT = waT.T @ xT, written to DRAM bounce buffer. Order pairs'
            # q/k d-tiles together (0,4,1,5,...) so pair p's attention inputs
            # are complete early and phase B can prefetch/start sooner.
            for dt in [0, 4, 1, 5, 2, 6, 3, 7]:
                wbs = []
                for ci in range(NCT):
                    wb = wblkp.tile([128, 128], F32R, tag="wblk", name=f"wb{dt}_{ci}")
                    nc.sync.dma_start(
                        wb[:],
                        waT[128 * ci : 128 * (ci + 1), 128 * dt : 128 * (dt + 1)],
                    )
                    wbs.append(wb)
                for jq in range(T // 512):
                    ps = ps_small.tile([128, 512], F32, tag="psA", name="psA")
                    for ci in range(NCT):
                        nc.tensor.matmul(
                            ps[:],
                            wbs[ci][:],
                            xt_t[ci][:, 512 * jq : 512 * (jq + 1)],
                            start=(ci == 0),
                            stop=(ci == NCT - 1),
                        )
                    st = stagep.tile([128, 512], F32R, tag="stage", name="stA")
                    nc.vector.tensor_copy(st[:], ps[:])
                    nc.sync.dma_start(
                        qk_dram[128 * dt : 128 * (dt + 1), 512 * jq : 512 * (jq + 1)],
                        st[:],
                    )

            # V = xT.T @ wv  (+ ones column per head)
            wv_t = []
            for ci in range(NCT):
                wv = wvp.tile([128, DL], F32R, tag="wv", name=f"wv{ci}")
                nc.sync.dma_start(wv[:], waT[128 * ci : 128 * (ci + 1), 2 * DL :])
                wv_t.append(wv)
            for tt in range(NKT):
                ps = ps_small.tile([128, 512], F32, tag="psA", name="psV")
                for ci in range(NCT):
                    nc.tensor.matmul(
                        ps[:],
                        xt_t[ci][:, 128 * tt : 128 * (tt + 1)],
                        wv_t[ci][:],
                        start=(ci == 0),
                        stop=(ci == NCT - 1),
                    )
                va = va_t[tt]
                va3 = va[:].rearrange("p (h d) -> p h d", d=HS + 1)
                ps3 = ps[:].rearrange("p (h d) -> p h d", d=HS)
                nc.vector.tensor_copy(va3[:, :, 0:HS], ps3[:])
                nc.vector.memset(va3[:, :, HS].bitcast(F32), 1.0)

            # ================= phase B: attention =================
            for p in range(4):  # head pairs
                qt = bigp.tile([128, T], F32R, tag="big", name=f"qt{p}")
                nc.sync.dma_start(qt[:], qk_dram[128 * p : 128 * (p + 1), :])
                kt = bigp.tile([128, T], F32R, tag="big", name=f"kt{p}")
                nc.sync.dma_start(kt[:], qk_dram[DL + 128 * p : DL + 128 * (p + 1), :])

                for j in range(NQC):
                    qs = slice(512 * j, 512 * (j + 1))
                    ytps = [
                        ps_yt.tile([HS + 1, 512], F32, tag="ytp", name="ytp0"),
                        ps_yt.tile([HS + 1, 512], F32, tag="ytp", name="ytp1"),
                    ]
                    n_kt = 4 * j + 4
                    # Software pipeline: S.T/exp run one k-tile ahead of the
                    # consuming y.T matmuls so the PE never waits on ACT.
                    ets = {}
                    for i in range(n_kt + 1):
                        if i < n_kt:
                            ks = slice(128 * i, 128 * (i + 1))
                            for h in range(2):  # head in pair, row-group packed
                                hp = slice(64 * h, 64 * (h + 1))
                                st = ps_st.tile(
                                    [128, 512], F32, tag="stp", name="stp"
                                )
                                nc.tensor.matmul(
                                    st[:], kt[hp, ks], qt[hp, qs],
                                    start=True, stop=True,
                                )
                                et = epool.tile([128, 512], F32R, tag="et", name="et")
                                nc.scalar.activation(
                                    et[:],
                                    st[:],
                                    mybir.ActivationFunctionType.Exp,
                                    scale=1.0 / math.sqrt(HS),
                                )
                                if i >= 4 * j:  # diagonal tile: causal mask
                                    nc.vector.tensor_mul(
                                        et[:], et[:], mask_t[i - 4 * j][:]
                                    )
                                ets[(i, h)] = et
                        ic = i - 1  # consume previous k-tile
                        if ic >= 0:
                            for h in range(2):
                                hh = 2 * p + h
                                nc.tensor.matmul(
                                    ytps[h][:],
                                    va_t[ic][:, 65 * hh : 65 * hh + 65],
                                    ets.pop((ic, h)),
                                    start=(ic == 0),
                                    stop=(ic == n_kt - 1),
                                )
                    for h in range(2):
                        # normalize: yT = y_unnorm * broadcast(1 / sum_k P)
                        rc = miscp.tile([1, 512], F32, tag="rc", name="rc")
                        nc.vector.reciprocal(rc[:], ytps[h][HS : HS + 1, :])
                        rb = miscp.tile([64, 512], F32, tag="rb", name="rb")
                        nc.gpsimd.partition_broadcast(rb[:], rc[:])
                        nc.vector.tensor_mul(
                            yt_t[p][64 * h : 64 * (h + 1), qs],
                            ytps[h][0:HS, :],
                            rb[:],
                        )

            # ================= phase C: output projection =================
            for e in range(C // 128):
                wps = []
                for p in range(4):
                    wp = wpep.tile([128, 128], F32R, tag=f"wpe{p}", name=f"wpe{p}")
                    nc.sync.dma_start(
                        wp[:],
                        wpT[128 * p : 128 * (p + 1), 128 * e : 128 * (e + 1)],
                    )
                    wps.append(wp)
                for jq in range(T // 512):
                    ps = ps_small.tile([128, 512], F32, tag="psA", name="psC")
                    for p in range(4):
                        nc.tensor.matmul(
                            ps[:],
                            wps[p][:],
                            yt_t[p][:, 512 * jq : 512 * (jq + 1)],
                            start=(p == 0),
                            stop=(p == 3),
                        )
                    ot = stagep.tile([128, 512], F32, tag="stage", name="stC")
                    nc.scalar.copy(ot[:], ps[:])
                    nc.sync.dma_start(
                        outT[128 * e : 128 * (e + 1), 512 * jq : 512 * (jq + 1)],
                        ot[:],
                    )

    nc.compile()
    return nc


_CACHE: dict = {}
_LAST_IN_MAPS = None


def _get_nc(T: int):
    if T not in _CACHE:
        _CACHE[T] = build(T)
    return _CACHE[T]


def _make_masks() -> np.ndarray:
    kk = np.arange(128)[:, None]
    qq = np.arange(512)[None, :]
    return np.stack(
        [(qq >= 128 * o + kk).astype(np.float32) for o in range(4)]
    )


def kernel(x: np.ndarray, w_attn: np.ndarray, w_proj: np.ndarray) -> np.ndarray:
    B, T, C_ = x.shape
    nc = _get_nc(T)
    masks = _make_masks()

    in_maps = []
    for core in range(8):
        b, g = core // 2, core % 2
        heads = range(8 * g, 8 * g + 8)
        rows = []
        for base in (0, C_, 2 * C_):  # q, k, v sections of w_attn
            for H in heads:
                rows.extend(range(base + 64 * H, base + 64 * H + 64))
        waT_l = np.ascontiguousarray(np.asarray(w_attn)[rows, :].T.astype(np.float32))
        dcols = [c for H in heads for c in range(64 * H, 64 * H + 64)]
        wpT_l = np.ascontiguousarray(np.asarray(w_proj)[:, dcols].T.astype(np.float32))
        xT_l = np.ascontiguousarray(np.asarray(x[b]).T.astype(np.float32))
        in_maps.append({"xT": xT_l, "waT": waT_l, "wpT": wpT_l, "masks": masks})

    global _LAST_IN_MAPS
    _LAST_IN_MAPS = in_maps
    res = bass_utils.run_bass_kernel_spmd(nc, in_maps, core_ids=list(range(8)))
    out = np.empty((B, T, C_), dtype=np.float32)
    for b in range(B):
        out[b] = (
            res.results[2 * b]["outT"].astype(np.float32)
            + res.results[2 * b + 1]["outT"].astype(np.float32)
        ).T
    return out
